# revision 3
# baseline (speedup 1.0000x reference)
"""Trainium2 Bass kernel for Qwen-style GQA attention block (B=2,S=2048,H=16,KV=8,D=128).

Sharding (8 cores): batch(2) x si-stripes(2) x head-half(2).
  core c: b=c>>2, sh=(c>>1)&1, hh=c&1
  - Q proj + attention for 8 q-heads (hh half) on 8 causally-balanced si blocks (sh stripes)
  - K/V proj for 4 kv heads over full S (replicated across the 2 stripe cores)
  - pair AllGather of ctx^T between the two head-half cores, then column-split o_proj.
All matmuls bf16 with fp32 PSUM accumulation. Softmax without max-subtraction
(scores are O(1) after QK RMSNorm); denominator via an appended ones-column on V.
"""
import sys

sys.path.insert(0, '/opt/trn_rl_repo')

import numpy as np

import concourse.bass as bass
import concourse.tile as tile
from concourse import mybir
from concourse.vector_clock import ScopedClock, VectorClock

B, S, HID = 2, 2048, 2048
H, KV, D = 16, 8, 128
EPS = 1e-6
SCALE = D ** -0.5
NBLK = S // 128  # 16
# causally balanced si-block stripes: sum(i+1) = 68 for both
MYBLKS = [[0, 2, 4, 6, 9, 11, 13, 15], [1, 3, 5, 7, 8, 10, 12, 14]]

F32 = mybir.dt.float32
BF16 = mybir.dt.bfloat16
AF = mybir.ActivationFunctionType


# ---------------------------------------------------------------------------
# Workarounds: this walrus supports only ONE sync-wait per instruction.
def _patched_drain_and_barrier(self, tick_clock, wait_clock):
    gc = tick_clock.global_clock
    vec = list(gc)
    nz = [i for i, v in enumerate(vec) if v > 0] or [0]
    for i in nz:
        cvec = [vec[j] if j == i else 0 for j in range(len(vec))]
        inst = self.nc.sync.drain()
        wait_clock.add_sem_waits(inst.ins, ScopedClock({None: VectorClock(cvec)}))
    self.nc.all_engine_barrier()
    assert self.sems is not None
    popped = self.nc._tile_sem_poison_stack.pop()
    assert popped is self._sem_poison
    self.nc.clear_and_free_semaphores(list(self.sems.allocated().values()))
    self.nc.all_engine_barrier()


tile.TileContext._drain_and_barrier = _patched_drain_and_barrier


def split_multi_waits(nc):
    for fn in nc.m.functions:
        for blk in fn.blocks:
            insts = list(blk.instructions)
            out = []
            changed = False
            for inst in insts:
                si = inst.sync_info
                if si is not None and len(si.on_wait) > 1:
                    waits = list(si.on_wait)
                    for k, w in enumerate(waits[:-1]):
                        out.append(mybir.InstNoOp(
                            name=f"{inst.name}.w{k}", engine=inst.engine,
                            sync_info=mybir.SyncInfo(on_wait=[w], on_update=[]),
                            text_hint="waitsplit"))
                    si.on_wait = [waits[-1]]
                    changed = True
                out.append(inst)
            if changed:
                blk.instructions[:] = out


# ---------------------------------------------------------------------------
def build_kernel():
    nc = bass.Bass(trn_type='TRN2')
    hT = nc.dram_tensor('hT', [HID, S], F32, kind='ExternalInput')
    qwT = nc.dram_tensor('qwT', [HID, 1024], F32, kind='ExternalInput')
    kwT = nc.dram_tensor('kwT', [HID, 512], F32, kind='ExternalInput')
    vwT = nc.dram_tensor('vwT', [HID, 512], F32, kind='ExternalInput')
    owT = nc.dram_tensor('owT', [2048, 1024], F32, kind='ExternalInput')
    # host-fused rope tables (cos/sin x norm-weight halves), [rows, 4, 64]
    qtab = nc.dram_tensor('qtab', [1024, 4, 64], F32, kind='ExternalInput')
    ktab = nc.dram_tensor('ktab', [S, 4, 64], F32, kind='ExternalInput')
    tri = nc.dram_tensor('tri', [128, 128], F32, kind='ExternalInput')
    iden = nc.dram_tensor('iden', [128, 128], F32, kind='ExternalInput')
    out_e = nc.dram_tensor('out', [1024, 1024], F32, kind='ExternalOutput')

    from contextlib import ExitStack
    with ExitStack() as ctx:
        tc = ctx.enter_context(tile.TileContext(nc))
        pool = lambda name, bufs, **kw: ctx.enter_context(
            tc.tile_pool(name=name, bufs=bufs, **kw))
        p_wq = pool('wq', 16)
        p_wk = pool('wk', 16)
        p_wv = pool('wv', 16)
        p_ht = pool('ht', 16)
        p_qt = pool('qt', 8)
        p_kt = pool('kt', 4)
        p_va = pool('va', 4)
        p_ctm = pool('ctm', 8)
        p_c = pool('const', 1)
        p_w = pool('work', 2)
        p_s = pool('small', 4)
        p_scl = pool('scl', 1)
        p_exp = pool('expb', 4)
        p_out = pool('outb', 2)
        ps_a = pool('psA', 2, space='PSUM')
        ps_s = pool('psS', 2, space='PSUM')
        ps_c = pool('psC', 2, space='PSUM')
        ps_t = pool('psT', 2, space='PSUM')
        p_d = pool('dram', 1, space='DRAM')
        if True:
            # ---- constants / weights (cast to bf16 on load) ----
            tri_s = p_c.tile([128, 128], BF16)
            nc.gpsimd.dma_start(tri_s[:], tri[:])
            iden_s = p_c.tile([128, 128], BF16)
            nc.gpsimd.dma_start(iden_s[:], iden[:])
            qtab_s = p_c.tile([128, 8, 4, 64], BF16)
            nc.gpsimd.dma_start(qtab_s[:], qtab.rearrange('(n p) t d -> p n t d', p=128))
            ktab_s = p_c.tile([128, 16, 4, 64], BF16)
            nc.gpsimd.dma_start(ktab_s[:], ktab.rearrange('(n p) t d -> p n t d', p=128))

            wq_s = [p_wq.tile([128, 1024], BF16, tag='wq', name='wq') for _ in range(16)]
            wk_s = [p_wk.tile([128, 512], BF16, tag='wk', name='wk') for _ in range(16)]
            wv_s = [p_wv.tile([128, 512], BF16, tag='wv', name='wv') for _ in range(16)]
            for ch in range(16):
                r = bass.ts(ch, 128)
                nc.gpsimd.dma_start(wq_s[ch][:], qwT[r, :])
                nc.gpsimd.dma_start(wk_s[ch][:], kwT[r, :])
                nc.gpsimd.dma_start(wv_s[ch][:], vwT[r, :])

            # persistent activation tiles
            QT = [p_qt.tile([128, 1024], BF16, tag='qt', name='qtl') for _ in range(8)]
            KT = [p_kt.tile([128, 2048], BF16, tag='kt', name='ktl') for _ in range(4)]
            VA = [p_va.tile([128, 16, 132], BF16, tag='va', name='va') for _ in range(4)]
            sclK = p_scl.tile([128, 16, 4], F32)   # SCALE * rstd_k per (sj_blk, kv)
            ctm = [p_ctm.tile([128, 1024], BF16, tag='ctm', name='ctm') for _ in range(8)]

            for kvh in range(4):  # ones column for the softmax denominator
                nc.gpsimd.memset(VA[kvh][:, :, 128:129], 1.0)

            bounds = [max(MYBLKS[0][bi], MYBLKS[1][bi]) for bi in range(8)]
            # per-core diagonal masks: dmask[bi][j] for j in {bounds[bi]-1, bounds[bi]}
            # encoded via a single input: dm [8, 2, 128, 128]
            dm = nc.dram_tensor('dm', [8, 2, 128, 128], F32, kind='ExternalInput')
            dm_s = p_c.tile([128, 8, 2, 128], BF16)
            nc.gpsimd.dma_start(dm_s[:], dm.rearrange('n t p d -> p n t d'))

            # ---- projections, two passes over s-halves ----
            for ph in range(2):
                ht_t = [p_ht.tile([128, 1024], BF16, tag='ht', name='ht') for _ in range(16)]
                for ch in range(16):
                    nc.gpsimd.dma_start(
                        ht_t[ch][:], hT[bass.ts(ch, 128), bass.ts(ph, 1024)])
                for j in range(8):
                    sb = ph * 8 + j
                    sslice = bass.ts(j, 128)
                    # ---- V ----
                    psV = ps_a.tile([128, 512], F32, tag='psA', name='psA')
                    for ch in range(16):
                        nc.tensor.matmul(psV[:], ht_t[ch][:, sslice], wv_s[ch][:],
                                         start=(ch == 0), stop=(ch == 15))
                    for kvh in range(4):
                        nc.scalar.copy(VA[kvh][:, sb, 0:128], psV[:, bass.ts(kvh, 128)])
                    # ---- K ----
                    psK = ps_a.tile([128, 512], F32, tag='psA', name='psA')
                    for ch in range(16):
                        nc.tensor.matmul(psK[:], ht_t[ch][:, sslice], wk_s[ch][:],
                                         start=(ch == 0), stop=(ch == 15))
                    kcp = p_w.tile([128, 512], F32, tag='kcp', name='kcp')
                    nc.scalar.copy(kcp[:], psK[:])
                    scr = p_w.tile([128, 512], F32, tag='scr', name='scr')
                    ss = p_s.tile([128, 4], F32, tag='ss', name='ss')
                    for kvh in range(4):
                        nc.scalar.activation(scr[:, bass.ts(kvh, 128)],
                                             kcp[:, bass.ts(kvh, 128)], AF.Square,
                                             accum_out=ss[:, kvh:kvh + 1])
                    nc.vector.tensor_scalar_add(ss[:], ss[:], float(EPS * D))
                    std = p_s.tile([128, 4], F32, tag='std', name='std')
                    nc.scalar.activation(std[:], ss[:], AF.Sqrt, scale=1.0 / D, bias=0.0)
                    rstd = p_s.tile([128, 4], F32, tag='rstd', name='rstd')
                    nc.vector.reciprocal(rstd[:], std[:])
                    nc.vector.tensor_scalar_mul(sclK[:, sb, :], rstd[:], SCALE)
                    # rope on raw K (w folded into ktab; rstd folded into exp scale)
                    kro = p_w.tile([128, 4, 128], BF16, tag='kro', name='kro')
                    lo = kcp[:].rearrange('p (t d) -> p t d', t=4)[:, :, 0:64]
                    hi = kcp[:].rearrange('p (t d) -> p t d', t=4)[:, :, 64:128]
                    tA = ktab_s[:, sb, :, :][:, 0:1, :]
                    tB = ktab_s[:, sb, :, :][:, 1:2, :]
                    tC = ktab_s[:, sb, :, :][:, 2:3, :]
                    tD = ktab_s[:, sb, :, :][:, 3:4, :]
                    t1 = p_w.tile([128, 4, 64], F32, tag='t1', name='t1')
                    t2 = p_w.tile([128, 4, 64], F32, tag='t2', name='t2')
                    mul_b(nc, t1[:], lo, tA)
                    mul_b(nc, t2[:], hi, tB)
                    nc.vector.tensor_sub(kro[:, :, 0:64], t1[:], t2[:])
                    mul_b(nc, t1[:], hi, tC)
                    mul_b(nc, t2[:], lo, tD)
                    nc.vector.tensor_add(kro[:, :, 64:128], t1[:], t2[:])
                    for kvh in range(4):  # transpose to KT
                        pst = ps_t.tile([128, 128], BF16, tag='psT', name='psT')
                        nc.tensor.transpose(pst[:], kro[:, kvh, :], iden_s[:])
                        nc.scalar.copy(KT[kvh][:, bass.ts(sb, 128)], pst[:])
                del ht_t

            # ---- Q projection from host-gathered hTq (my si rows, local order) ----
            hTq = nc.dram_tensor('hTq', [HID, 1024], F32, kind='ExternalInput')
            htq_t = [p_ht.tile([128, 1024], BF16, tag='ht', name='ht') for _ in range(16)]
            for ch in range(16):
                nc.gpsimd.dma_start(htq_t[ch][:], hTq[bass.ts(ch, 128), :])
            for bi in range(8):
                sslice = bass.ts(bi, 128)
                for qg in range(2):
                    psQ = ps_a.tile([128, 512], F32, tag='psA', name='psA')
                    for ch in range(16):
                        nc.tensor.matmul(psQ[:], htq_t[ch][:, sslice],
                                         wq_s[ch][:, bass.ts(qg, 512)],
                                         start=(ch == 0), stop=(ch == 15))
                    qcp = p_w.tile([128, 512], F32, tag='kcp', name='qcp')
                    nc.scalar.copy(qcp[:], psQ[:])
                    scr = p_w.tile([128, 512], F32, tag='scr', name='scr')
                    ss = p_s.tile([128, 4], F32, tag='ss', name='ss')
                    for hq in range(4):
                        nc.scalar.activation(scr[:, bass.ts(hq, 128)],
                                             qcp[:, bass.ts(hq, 128)], AF.Square,
                                             accum_out=ss[:, hq:hq + 1])
                    nc.vector.tensor_scalar_add(ss[:], ss[:], float(EPS * D))
                    std = p_s.tile([128, 4], F32, tag='std', name='std')
                    nc.scalar.activation(std[:], ss[:], AF.Sqrt, scale=1.0 / D, bias=0.0)
                    rstd = p_s.tile([128, 4], F32, tag='rstd', name='rstd')
                    nc.vector.reciprocal(rstd[:], std[:])
                    qro = p_w.tile([128, 4, 128], BF16, tag='kro', name='kro')
                    lo = qcp[:].rearrange('p (t d) -> p t d', t=4)[:, :, 0:64]
                    hi = qcp[:].rearrange('p (t d) -> p t d', t=4)[:, :, 64:128]
                    tA = qtab_s[:, bi, :, :][:, 0:1, :]
                    tB = qtab_s[:, bi, :, :][:, 1:2, :]
                    tC = qtab_s[:, bi, :, :][:, 2:3, :]
                    tD = qtab_s[:, bi, :, :][:, 3:4, :]
                    t1 = p_w.tile([128, 4, 64], F32, tag='t1', name='t1')
                    t2 = p_w.tile([128, 4, 64], F32, tag='t2', name='t2')
                    mul_b(nc, t1[:], lo, tA)
                    mul_b(nc, t2[:], hi, tB)
                    nc.vector.tensor_sub(qro[:, :, 0:64], t1[:], t2[:])
                    mul_b(nc, t1[:], hi, tC)
                    mul_b(nc, t2[:], lo, tD)
                    nc.vector.tensor_add(qro[:, :, 64:128], t1[:], t2[:])
                    qn = p_w.tile([128, 4, 128], BF16, tag='qn', name='qn')
                    for hq in range(4):
                        nc.vector.tensor_scalar_mul(qn[:, hq, :], qro[:, hq, :],
                                                    rstd[:, hq:hq + 1])
                        pst = ps_t.tile([128, 128], BF16, tag='psT', name='psT')
                        nc.tensor.transpose(pst[:], qn[:, hq, :], iden_s[:])
                        nc.scalar.copy(QT[qg * 4 + hq][:, bass.ts(bi, 128)], pst[:])

            # ---- attention ----
            for h in range(8):
                kvh = h // 2
                for bi in range(8):
                    gi = bounds[bi]
                    psC = ps_c.tile([128, 132], F32, tag='psC', name='psC')
                    for j in range(gi + 1):
                        psS = ps_s.tile([128, 128], F32, tag='psS', name='psS')
                        nc.tensor.matmul(psS[:], KT[kvh][:, bass.ts(j, 128)],
                                         QT[h][:, bass.ts(bi, 128)],
                                         start=True, stop=True)
                        ex = p_exp.tile([128, 128], BF16, tag='expb', name='expb')
                        nc.scalar.activation(ex[:], psS[:], AF.Exp,
                                             scale=sclK[:, j, kvh:kvh + 1])
                        if j >= gi - 1:  # possible diagonal/overhang: apply mask
                            nc.vector.tensor_mul(ex[:], ex[:], dm_s[:, bi, j - (gi - 1), :])
                        nc.tensor.matmul(psC[:, 0:129], ex[:], VA[kvh][:, j, 0:129],
                                         start=(j == 0), stop=(j == gi))
                    rd = p_s.tile([128, 1], F32, tag='rd', name='rd')
                    nc.vector.reciprocal(rd[:], psC[:, 128:129])
                    cn = p_w.tile([128, 128], BF16, tag='cn', name='cn')
                    nc.vector.tensor_scalar_mul(cn[:], psC[:, 0:128], rd[:])
                    pst = ps_t.tile([128, 128], BF16, tag='psT', name='psT')
                    nc.tensor.transpose(pst[:], cn[:], iden_s[:])
                    nc.scalar.copy(ctm[h][:, bass.ts(bi, 128)], pst[:])

            wo_s = [p_ht.tile([128, 1024], BF16, tag='ht', name='wo') for _ in range(16)]
            for ch in range(16):
                nc.gpsimd.dma_start(wo_s[ch][:], owT[bass.ts(ch, 128), :])

            # ---- pair AllGather of ctx^T ----
            cc_in = p_d.tile([1024, 1024], BF16)
            cc_out = p_d.tile([2048, 1024], BF16)
            for h in range(8):
                nc.sync.dma_start(cc_in[bass.ts(h, 128), :], ctm[h][:])
            nc.gpsimd.collective_compute(
                'AllGather', mybir.AluOpType.bypass,
                replica_groups=[[0, 1], [2, 3], [4, 5], [6, 7]],
                ins=[cc_in.opt()], outs=[cc_out.opt()])
            ctf = [p_wq.tile([128, 1024], BF16, tag='wq', name='ctf') for _ in range(16)]
            for ch in range(16):
                nc.sync.dma_start(ctf[ch][:], cc_out[bass.ts(ch, 128), :])

            # ---- o_proj (my ho half columns) ----
            for bi in range(8):
                for nt in range(2):
                    psO = ps_a.tile([128, 512], F32, tag='psA', name='psA')
                    for ch in range(16):
                        nc.tensor.matmul(psO[:], ctf[ch][:, bass.ts(bi, 128)],
                                         wo_s[ch][:, bass.ts(nt, 512)],
                                         start=(ch == 0), stop=(ch == 15))
                    ob = p_out.tile([128, 512], F32, tag='outb', name='outb')
                    nc.scalar.copy(ob[:], psO[:])
                    nc.sync.dma_start(out_e[bass.ts(bi, 128), bass.ts(nt, 512)], ob[:])

    split_multi_waits(nc)
    return nc


def mul_b(nc, out, a, b):
    """tensor_tensor multiply with free-dim broadcast of b over dim 1."""
    a2, b2 = bass.broadcast_tensor_aps(a, b)
    nc.vector.tensor_mul(out, a2, b2)


# ---------------------------------------------------------------------------
_NC_CACHE = None
_LAST_IN_MAPS = None


def _get_nc():
    global _NC_CACHE
    if _NC_CACHE is None:
        _NC_CACHE = build_kernel()
    return _NC_CACHE


def kernel(hidden_states, cos, sin, q_w, k_w, v_w, o_w, q_norm_w, k_norm_w):
    from concourse.bass_utils import run_bass_kernel_spmd

    hidden_states = np.asarray(hidden_states, np.float32)
    cos = np.asarray(cos, np.float32)
    sin = np.asarray(sin, np.float32)
    q_w = np.asarray(q_w, np.float32)
    k_w = np.asarray(k_w, np.float32)
    v_w = np.asarray(v_w, np.float32)
    o_w = np.asarray(o_w, np.float32)
    q_norm_w = np.asarray(q_norm_w, np.float32)
    k_norm_w = np.asarray(k_norm_w, np.float32)

    tri_np = np.triu(np.ones((128, 128), np.float32))  # [sj,si]: valid sj<=si
    iden_np = np.eye(128, dtype=np.float32)

    def rope_tabs(c, s_, w):
        # tables [rows, 4, 64]: A=c_lo*w_lo, B=s_lo*w_hi, C=c_lo*w_hi, D=s_lo*w_lo
        cl, sl = c[:, 0:64], s_[:, 0:64]
        wl, wh = w[0:64], w[64:128]
        return np.stack([cl * wl, sl * wh, cl * wh, sl * wl], axis=1).astype(np.float32)

    bounds = [max(MYBLKS[0][bi], MYBLKS[1][bi]) for bi in range(8)]

    in_maps = []
    for c in range(8):
        b, sh, hh = c >> 2, (c >> 1) & 1, c & 1
        blks = MYBLKS[sh]
        rows = np.concatenate([np.arange(g * 128, (g + 1) * 128) for g in blks])
        hT = np.ascontiguousarray(hidden_states[b].T)
        hTq = np.ascontiguousarray(hidden_states[b][rows].T)
        qwT = np.ascontiguousarray(q_w[hh * 1024:(hh + 1) * 1024].T)
        kwT = np.ascontiguousarray(k_w[hh * 512:(hh + 1) * 512].T)
        vwT = np.ascontiguousarray(v_w[hh * 512:(hh + 1) * 512].T)
        owT = np.ascontiguousarray(o_w[hh * 1024:(hh + 1) * 1024].T)
        qtab = rope_tabs(cos[b][rows], sin[b][rows], q_norm_w)
        ktab = rope_tabs(cos[b], sin[b], k_norm_w)
        # diagonal masks dm[bi, t]: t=0 -> sj block gi-1, t=1 -> sj block gi
        # my true causal diagonal is at block g=blks[bi] (<= bounds[bi]).
        dm = np.zeros((8, 2, 128, 128), np.float32)
        for bi in range(8):
            g, gb = blks[bi], bounds[bi]
            for t, j in enumerate((gb - 1, gb)):
                if j < 0:
                    continue
                if j < g:
                    dm[bi, t] = 1.0
                elif j == g:
                    dm[bi, t] = tri_np
                # j > g: stays 0 (block fully masked)
        in_maps.append(dict(
            hT=hT, hTq=hTq, qwT=qwT, kwT=kwT, vwT=vwT, owT=owT,
            qtab=qtab, ktab=ktab, tri=tri_np, iden=iden_np, dm=dm))

    global _LAST_IN_MAPS
    _LAST_IN_MAPS = in_maps
    nc = _get_nc()
    res = run_bass_kernel_spmd(nc, in_maps, core_ids=list(range(8)))

    out = np.zeros((B, S, HID), np.float32)
    for c in range(8):
        b, sh, hh = c >> 2, (c >> 1) & 1, c & 1
        o = res.results[c]['out']  # [1024, 1024]
        for bi, g in enumerate(MYBLKS[sh]):
            out[b, g * 128:(g + 1) * 128, hh * 1024:(hh + 1) * 1024] = \
                o[bi * 128:(bi + 1) * 128]
    return out


if __name__ == '__main__':
    sys.path.insert(0, '/root/problem')
    import reference
    inputs = {k: np.asarray(v) for k, v in reference.setup_inputs().items()}
    exp = np.asarray(reference.reference(**inputs))
    act = kernel(**inputs)
    err = np.abs(act - exp)
    rel = np.linalg.norm(act - exp) / np.linalg.norm(exp)
    print('Relative error:', rel, 'max abs err:', err.max())



# revision 18
# speedup vs baseline: 2.0230x; 2.0230x over previous
"""Trainium2 Bass kernel for Qwen-style GQA attention block (B=2,S=2048,H=16,KV=8,D=128).

Sharding (8 cores): batch(2) x si-stripes(2) x head-half(2).
  core c: b=c>>2, sh=(c>>1)&1, hh=c&1
  - each core projects Q/K/V for ITS stripe rows only (1024 tokens); K/V results
    (roped, transposed, normalized) are exchanged between the two stripe cores
    via a small AllGather so both see full-S K/V.
  - attention j-outer with wide score tiles (stationary K-block reuse, wide exp).
  - pair AllGather of ctx^T split in two head-groups, column-split o_proj in two
    passes so the second collective hides under the first o_proj pass.
All matmuls bf16 with fp32 PSUM accumulation. Softmax without max-subtraction
(scores are O(1) after QK RMSNorm); denominator via an appended ones-column on V.
"""
import sys

sys.path.insert(0, '/opt/trn_rl_repo')

import numpy as np

import concourse.bass as bass
import concourse.tile as tile
from concourse import mybir
from concourse.vector_clock import ScopedClock, VectorClock

B, S, HID = 2, 2048, 2048
H, KV, D = 16, 8, 128
EPS = 1e-6
SCALE = D ** -0.5
NBLK = S // 128  # 16
# causally balanced si-block stripes: sum(i+1) = 68 for both
MYBLKS = [[0, 2, 4, 6, 9, 11, 13, 15], [1, 3, 5, 7, 8, 10, 12, 14]]
BOUNDS = [max(MYBLKS[0][bi], MYBLKS[1][bi]) for bi in range(8)]  # [1,3,..,15]
# o_proj ctx row order after the two pair-AllGathers (global head ids)
OHEAD_ORDER = [0, 1, 2, 3, 8, 9, 10, 11, 4, 5, 6, 7, 12, 13, 14, 15]

F32 = mybir.dt.float32
BF16 = mybir.dt.bfloat16
AF = mybir.ActivationFunctionType
MUL = mybir.AluOpType.mult
ADD = mybir.AluOpType.add


# ---------------------------------------------------------------------------
# Workarounds: this walrus supports only ONE sync-wait per instruction.
def _patched_drain_and_barrier(self, tick_clock, wait_clock):
    gc = tick_clock.global_clock
    vec = list(gc)
    nz = [i for i, v in enumerate(vec) if v > 0] or [0]
    for i in nz:
        cvec = [vec[j] if j == i else 0 for j in range(len(vec))]
        inst = self.nc.sync.drain()
        wait_clock.add_sem_waits(inst.ins, ScopedClock({None: VectorClock(cvec)}))
    self.nc.all_engine_barrier()
    assert self.sems is not None
    popped = self.nc._tile_sem_poison_stack.pop()
    assert popped is self._sem_poison
    self.nc.clear_and_free_semaphores(list(self.sems.allocated().values()))
    self.nc.all_engine_barrier()


tile.TileContext._drain_and_barrier = _patched_drain_and_barrier


def split_multi_waits(nc):
    for fn in nc.m.functions:
        for blk in fn.blocks:
            insts = list(blk.instructions)
            out = []
            changed = False
            for inst in insts:
                si = inst.sync_info
                if si is not None and len(si.on_wait) > 1:
                    waits = list(si.on_wait)
                    for k, w in enumerate(waits[:-1]):
                        out.append(mybir.InstNoOp(
                            name=f"{inst.name}.w{k}", engine=inst.engine,
                            sync_info=mybir.SyncInfo(on_wait=[w], on_update=[]),
                            text_hint="waitsplit"))
                    si.on_wait = [waits[-1]]
                    changed = True
                out.append(inst)
            if changed:
                blk.instructions[:] = out


# ---------------------------------------------------------------------------
def build_kernel():
    nc = bass.Bass(trn_type='TRN2')
    # hidden^T for THIS core's stripe rows (local bi-block order)
    hT = nc.dram_tensor('hT', [HID, 1024], F32, kind='ExternalInput')
    qwT = nc.dram_tensor('qwT', [HID, 1024], F32, kind='ExternalInput')
    kwT = nc.dram_tensor('kwT', [HID, 512], F32, kind='ExternalInput')
    vwT = nc.dram_tensor('vwT', [HID, 512], F32, kind='ExternalInput')
    owT = nc.dram_tensor('owT', [2048, 1024], F32, kind='ExternalInput')
    # host-fused rope tables (cos/sin x norm-weight halves), [1024, 4, 64]
    qtab = nc.dram_tensor('qtab', [1024, 4, 64], F32, kind='ExternalInput')
    ktab = nc.dram_tensor('ktab', [1024, 4, 64], F32, kind='ExternalInput')
    iden = nc.dram_tensor('iden', [128, 128], F32, kind='ExternalInput')
    # per-core diagonal masks dm[bi, t] for j in {BOUNDS[bi]-1, BOUNDS[bi]}
    dm = nc.dram_tensor('dm', [8, 2, 128, 128], F32, kind='ExternalInput')
    out_e = nc.dram_tensor('out', [1024, 1024], F32, kind='ExternalOutput')

    from contextlib import ExitStack
    with ExitStack() as ctx:
        tc = ctx.enter_context(tile.TileContext(nc))
        pool = lambda name, bufs, **kw: ctx.enter_context(
            tc.tile_pool(name=name, bufs=bufs, **kw))
        p_c = pool('const', 1)
        p_wv = pool('wv', 4)      # wv tiles, later QT tiles
        p_wk = pool('wk', 4)      # wk tiles, later ctm tiles
        p_wq = pool('wq', 4)      # wq tiles, later wo tiles
        p_h = pool('ht', 4)       # hT tiles, later ctf tiles
        p_kv = pool('kvstage', 1)  # KTh/VAh staging + KT_all/VA_all
        p_w = pool('work', 2)     # rope scratch etc
        p_s = pool('small', 4)
        p_ex = pool('expb', 4)
        p_ob = pool('outb', 2)
        ps_m = pool('psM', 3, space='PSUM')   # proj psums + scores + o_proj
        ps_c = pool('psC', 1, space='PSUM')   # ctx accumulators (3 tags)
        ps_t = pool('psT', 2, space='PSUM')   # transposes
        p_d = pool('dram', 1, space='DRAM')

        # ---- DMA prologue (gpsimd queue): wv/hT first so V proj starts early
        wv_s = [p_wv.tile([128, 4, 512], BF16, tag='wv', name='wv') for _ in range(4)]
        wk_s = [p_wk.tile([128, 4, 512], BF16, tag='wk', name='wk') for _ in range(4)]
        wq_s = [p_wq.tile([128, 4, 1024], BF16, tag='wq', name='wq') for _ in range(4)]
        ht_t = [p_h.tile([128, 4, 1024], BF16, tag='ht', name='ht') for _ in range(4)]
        for g in range(4):
            r = bass.ts(g, 512)
            nc.gpsimd.dma_start(wv_s[g][:], vwT[r, :].rearrange('(n p) c -> p n c', p=128))
            nc.gpsimd.dma_start(ht_t[g][:], hT[r, :].rearrange('(n p) c -> p n c', p=128))
        for g in range(4):
            nc.gpsimd.dma_start(wk_s[g][:], kwT[bass.ts(g, 512), :].rearrange('(n p) c -> p n c', p=128))
        iden_s = p_c.tile([128, 128], BF16)
        nc.gpsimd.dma_start(iden_s[:], iden[:])
        ktab_s = p_c.tile([128, 8, 4, 64], BF16)
        nc.gpsimd.dma_start(ktab_s[:], ktab.rearrange('(n p) t d -> p n t d', p=128))
        qtab_s = p_c.tile([128, 8, 4, 64], BF16)
        nc.gpsimd.dma_start(qtab_s[:], qtab.rearrange('(n p) t d -> p n t d', p=128))
        for g in range(4):
            nc.gpsimd.dma_start(wq_s[g][:], qwT[bass.ts(g, 512), :].rearrange('(n p) c -> p n c', p=128))
        dm_s = p_c.tile([128, 8, 2, 128], BF16)
        nc.gpsimd.dma_start(dm_s[:], dm.rearrange('n t p d -> p n t d'))

        # persistent K/V stores (full S, post-exchange) + local staging
        KTh = p_kv.tile([128, 4, 1024], BF16, tag='kth', name='KTh')
        VAh = p_kv.tile([128, 4, 8, 132], BF16, tag='vah', name='VAh')
        KT = p_kv.tile([128, 4, 2, 1024], BF16, tag='kt', name='KT')
        VA = p_kv.tile([128, 4, 16, 132], BF16, tag='va', name='VA')
        nc.gpsimd.memset(VAh[:, :, :, 128:132], 1.0)

        # ---- V projection (my stripe rows; sb = local block) ----
        for sb in range(8):
            psV = ps_m.tile([128, 512], F32, tag='ps', name='psV')
            for ch in range(16):
                nc.tensor.matmul(psV[:], ht_t[ch // 4][:, ch % 4, bass.ts(sb, 128)],
                                 wv_s[ch // 4][:, ch % 4, :],
                                 start=(ch == 0), stop=(ch == 15))
            nc.scalar.copy(VAh[:, :, sb, 0:128],
                           psV[:].rearrange('p (k d) -> p k d', k=4))

        # ---- K projection + RMSNorm(*SCALE) + rope + transpose ----
        for sb in range(8):
            psK = ps_m.tile([128, 512], F32, tag='ps', name='psK')
            for ch in range(16):
                nc.tensor.matmul(psK[:], ht_t[ch // 4][:, ch % 4, bass.ts(sb, 128)],
                                 wk_s[ch // 4][:, ch % 4, :],
                                 start=(ch == 0), stop=(ch == 15))
            kraw = p_w.tile([128, 4, 128], BF16, tag='raw', name='kraw')
            nc.vector.tensor_scalar_add(kraw[:], psK[:].rearrange('p (k d) -> p k d', k=4), 0.0)
            sqd = p_w.tile([128, 4, 128], BF16, tag='sqd', name='sqd')
            nc.vector.tensor_mul(sqd[:], kraw[:], kraw[:])
            ms = p_s.tile([128, 4], F32, tag='ms', name='ms')
            nc.vector.tensor_reduce(ms[:], sqd[:], mybir.AxisListType.X, ADD)
            nc.vector.tensor_scalar_add(ms[:], ms[:], float(EPS * D))
            std = p_s.tile([128, 4], F32, tag='std', name='std')
            nc.scalar.activation(std[:], ms[:], AF.Sqrt, scale=1.0 / D, bias=0.0)
            rstd = p_s.tile([128, 4], F32, tag='rstd', name='rstd')
            nc.vector.reciprocal(rstd[:], std[:])
            rstdS = p_s.tile([128, 4], F32, tag='rstds', name='rstdS')
            nc.vector.tensor_scalar_mul(rstdS[:], rstd[:], SCALE)
            kcs = p_w.tile([128, 4, 128], BF16, tag='kcs', name='kcs')
            for kvh in range(4):
                nc.vector.tensor_scalar_mul(kcs[:, kvh, :], kraw[:, kvh, :],
                                            rstdS[:, kvh:kvh + 1])
            # rope on gpsimd (keeps DVE free); tables already fold k_norm_w
            lo, hi = kcs[:, :, 0:64], kcs[:, :, 64:128]
            tA = ktab_s[:, sb, :, :][:, 0:1, :]
            tB = ktab_s[:, sb, :, :][:, 1:2, :]
            tC = ktab_s[:, sb, :, :][:, 2:3, :]
            tD = ktab_s[:, sb, :, :][:, 3:4, :]
            t_ = p_w.tile([128, 4, 4, 64], BF16, tag='t4', name='t4')
            kro = p_w.tile([128, 4, 128], BF16, tag='kro', name='kro')
            mul_b(nc.gpsimd, t_[:, 0], lo, tA)
            mul_b(nc.gpsimd, t_[:, 1], hi, tB)
            nc.gpsimd.tensor_sub(kro[:, :, 0:64], t_[:, 0], t_[:, 1])
            mul_b(nc.gpsimd, t_[:, 2], hi, tC)
            mul_b(nc.gpsimd, t_[:, 3], lo, tD)
            nc.gpsimd.tensor_add(kro[:, :, 64:128], t_[:, 2], t_[:, 3])
            psTk = ps_t.tile([128, 4, 128], BF16, tag='pst', name='psTk')
            for kvh in range(4):
                nc.tensor.transpose(psTk[:, kvh, :], kro[:, kvh, :], iden_s[:])
            nc.scalar.copy(KTh[:, :, bass.ts(sb, 128)], psTk[:])

        # ---- exchange K/V halves between the stripe pair (hidden under Q) ----
        cckv_in = p_d.tile([128, 8320], BF16, tag='cckvi', name='cckv_in')
        cckv_out = p_d.tile([256, 8320], BF16, tag='cckvo', name='cckv_out')
        nc.sync.dma_start(cckv_in[:, 0:4096], KTh[:])
        nc.sync.dma_start(cckv_in[:, 4096:8320], VAh[:])
        nc.gpsimd.collective_compute(
            'AllGather', mybir.AluOpType.bypass,
            replica_groups=[[0, 2], [1, 3], [4, 6], [5, 7]],
            ins=[cckv_in.opt()], outs=[cckv_out.opt()])
        # read back both stripes, kept in stripe-local order:
        # global block j lives at (rank r_j, slot j//2), r_j = (j%2) ^ (j>=8)
        for r in range(2):
            src = cckv_out[bass.ts(r, 128), :]
            nc.sync.dma_start(KT[:, :, r, :],
                              src[:, 0:4096].rearrange('p (k c) -> p k c', k=4))
            nc.sync.dma_start(VA[:, :, r * 8:(r + 1) * 8, :].rearrange('p k s w -> p k (s w)'),
                              src[:, 4096:8320].rearrange('p (k x) -> p k x', k=4))

        # ---- Q projection (PE busy while exchange completes) ----
        QT = [p_wv.tile([128, 2, 1024], BF16, tag='wv', name='QT') for _ in range(4)]
        for bi in range(8):
            for qg in range(2):
                psQ = ps_m.tile([128, 512], F32, tag='ps', name='psQ')
                for ch in range(16):
                    nc.tensor.matmul(psQ[:], ht_t[ch // 4][:, ch % 4, bass.ts(bi, 128)],
                                     wq_s[ch // 4][:, ch % 4, bass.ts(qg, 512)],
                                     start=(ch == 0), stop=(ch == 15))
                qraw = p_w.tile([128, 4, 128], BF16, tag='raw', name='qraw')
                nc.vector.tensor_scalar_add(qraw[:], psQ[:].rearrange('p (k d) -> p k d', k=4), 0.0)
                sqd = p_w.tile([128, 4, 128], BF16, tag='sqd', name='sqd')
                nc.vector.tensor_mul(sqd[:], qraw[:], qraw[:])
                ms = p_s.tile([128, 4], F32, tag='ms', name='ms')
                nc.vector.tensor_reduce(ms[:], sqd[:], mybir.AxisListType.X, ADD)
                nc.vector.tensor_scalar_add(ms[:], ms[:], float(EPS * D))
                std = p_s.tile([128, 4], F32, tag='std', name='std')
                nc.scalar.activation(std[:], ms[:], AF.Sqrt, scale=1.0 / D, bias=0.0)
                rstd = p_s.tile([128, 4], F32, tag='rstd', name='rstd')
                nc.vector.reciprocal(rstd[:], std[:])
                qcs = p_w.tile([128, 4, 128], BF16, tag='kcs', name='qcs')
                for hq in range(4):
                    nc.vector.tensor_scalar_mul(qcs[:, hq, :], qraw[:, hq, :],
                                                rstd[:, hq:hq + 1])
                lo, hi = qcs[:, :, 0:64], qcs[:, :, 64:128]
                tA = qtab_s[:, bi, :, :][:, 0:1, :]
                tB = qtab_s[:, bi, :, :][:, 1:2, :]
                tC = qtab_s[:, bi, :, :][:, 2:3, :]
                tD = qtab_s[:, bi, :, :][:, 3:4, :]
                t_ = p_w.tile([128, 4, 4, 64], BF16, tag='t4', name='t4')
                qro = p_w.tile([128, 4, 128], BF16, tag='kro', name='qro')
                mul_b(nc.vector, t_[:, 0], lo, tA)
                mul_b(nc.vector, t_[:, 1], hi, tB)
                nc.vector.tensor_sub(qro[:, :, 0:64], t_[:, 0], t_[:, 1])
                mul_b(nc.vector, t_[:, 2], hi, tC)
                mul_b(nc.vector, t_[:, 3], lo, tD)
                nc.vector.tensor_add(qro[:, :, 64:128], t_[:, 2], t_[:, 3])
                psTq = ps_t.tile([128, 4, 128], BF16, tag='pst', name='psTq')
                for hq in range(4):
                    nc.tensor.transpose(psTq[:, hq, :], qro[:, hq, :], iden_s[:])
                nc.scalar.copy(QT[qg * 2][:, :, bass.ts(bi, 128)], psTq[:, 0:2, :])
                nc.scalar.copy(QT[qg * 2 + 1][:, :, bass.ts(bi, 128)], psTq[:, 2:4, :])

        # wo loads (reuse wq slots; runs during attention)
        wo_s = [p_wq.tile([128, 4, 1024], BF16, tag='wq', name='wo') for _ in range(4)]
        for g in range(4):
            nc.gpsimd.dma_start(wo_s[g][:], owT[bass.ts(g, 512), :].rearrange('(n p) c -> p n c', p=128))

        # ---- attention: j-outer, wide score tiles ----
        ctm = [p_wk.tile([128, 2, 1024], BF16, tag='wk', name='ctm') for _ in range(4)]
        # psC slot layout: tag -> (first bi, n slots)
        CGRP = [(0, 3), (3, 3), (6, 2)]

        def cslot(pc, bi):
            g = 0 if bi < 3 else (1 if bi < 6 else 2)
            s = bi - CGRP[g][0]
            return pc[g][:, s * 132:s * 132 + 129]

        for h in range(8):
            kvh = h // 2
            qt_ap = QT[h // 2][:, h % 2, :]
            pc = [ps_c.tile([128, CGRP[g][1] * 132], F32, tag=f'c{g}', name=f'pc{g}')
                  for g in range(3)]
            exs = {}
            psTc = None
            for it in range(17):
                # QK for j=it (chunks), exp, mask
                if it < 16:
                    j = it
                    b0 = j // 2
                    if b0 < 4:
                        chunks = [(b0 * 128, 512), (512, 1024)]
                    else:
                        chunks = [(b0 * 128, 1024)]
                    rj = (j % 2) ^ (1 if j >= 8 else 0)
                    kt_ap = KT[:, kvh, rj, bass.ts(j // 2, 128)]
                    cur = []
                    for (c0, c1) in chunks:
                        w = c1 - c0
                        psS = ps_m.tile([128, 512], F32, tag='ps', name='psS')
                        nc.tensor.matmul(psS[:, 0:w], kt_ap, qt_ap[:, c0:c1],
                                         start=True, stop=True)
                        ex = p_ex.tile([128, 512], BF16, tag='ex', name='ex')
                        nc.scalar.activation(ex[:, 0:w], psS[:, 0:w], AF.Exp)
                        cur.append((c0, c1, ex))
                    # diagonal/overhang mask: block bi=b0, t = j parity
                    nc.gpsimd.tensor_mul(cur[0][2][:, 0:128], cur[0][2][:, 0:128],
                                         dm_s[:, b0, j % 2, :])
                    exs[j] = cur
                # PV for j=it-1 (+finalizations)
                if it > 0:
                    j = it - 1
                    b0 = j // 2
                    rj = (j % 2) ^ (1 if j >= 8 else 0)
                    va_ap = VA[:, kvh, rj * 8 + j // 2, 0:129]
                    for bi in range(b0, 8):
                        (c0, c1, ex) = exs[j][0] if bi * 128 < exs[j][0][1] else exs[j][1]
                        exsub = ex[:, bi * 128 - c0:bi * 128 - c0 + 128]
                        # start=True clears has_written for the WHOLE bank: issue it
                        # only on the bank's first matmul; other slots first-write
                        # via the overwrite-where-unwritten path.
                        nc.tensor.matmul(cslot(pc, bi), exsub, va_ap,
                                         start=(j == 0 and bi in (0, 3, 6)),
                                         stop=(j == BOUNDS[bi]),
                                         skip_group_check=True)
                    del exs[j]
                    # finalize bi whose last block was j
                    if it % 2 == 0:
                        bi = (it - 2) // 2
                        sl = cslot(pc, bi)
                        rd = p_s.tile([128, 1], F32, tag='rd', name='rd')
                        nc.vector.reciprocal(rd[:], sl[:, 128:129])
                        cn = p_w.tile([128, 128], BF16, tag='cn', name='cn')
                        nc.vector.tensor_scalar_mul(cn[:], sl[:, 0:128], rd[:])
                        if psTc is None:
                            psTc = ps_t.tile([128, 4, 128], BF16, tag='pst', name='psTc')
                        nc.tensor.transpose(psTc[:, bi % 2, :], cn[:], iden_s[:])
                        if bi % 2 == 1:
                            nc.vector.tensor_scalar_add(
                                ctm[h // 2][:, h % 2, bass.ts(bi // 2, 256)],
                                psTc[:, 0:2, :], 0.0)
                            psTc = None
            # after heads 0-3: first ctx pair-AllGather (hidden under h4-7)
            if h == 3:
                cc0_in = p_d.tile([512, 1024], BF16, tag='cc0i', name='cc0_in')
                cc0_out = p_d.tile([1024, 1024], BF16, tag='cc0o', name='cc0_out')
                for i in range(2):
                    nc.sync.dma_start(
                        cc0_in[bass.ts(i, 256), :].rearrange('(c p) s -> p c s', c=2),
                        ctm[i][:])
                nc.gpsimd.collective_compute(
                    'AllGather', mybir.AluOpType.bypass,
                    replica_groups=[[0, 1], [2, 3], [4, 5], [6, 7]],
                    ins=[cc0_in.opt()], outs=[cc0_out.opt()])
                ctf0 = [p_h.tile([128, 2, 1024], BF16, tag='ht', name='ctf0')
                        for _ in range(4)]
                for i in range(4):
                    nc.sync.dma_start(
                        ctf0[i][:],
                        cc0_out[bass.ts(i, 256), :].rearrange('(c p) s -> p c s', c=2))

        # second ctx AllGather (transfers while o_proj pass 0 runs)
        cc1_in = p_d.tile([512, 1024], BF16, tag='cc1i', name='cc1_in')
        cc1_out = p_d.tile([1024, 1024], BF16, tag='cc1o', name='cc1_out')
        for i in range(2):
            nc.sync.dma_start(
                cc1_in[bass.ts(i, 256), :].rearrange('(c p) s -> p c s', c=2),
                ctm[i + 2][:])
        nc.gpsimd.collective_compute(
            'AllGather', mybir.AluOpType.bypass,
            replica_groups=[[0, 1], [2, 3], [4, 5], [6, 7]],
            ins=[cc1_in.opt()], outs=[cc1_out.opt()])
        ctf1 = [p_wv.tile([128, 2, 1024], BF16, tag='wv', name='ctf1') for _ in range(4)]
        for i in range(4):
            nc.sync.dma_start(
                ctf1[i][:],
                cc1_out[bass.ts(i, 256), :].rearrange('(c p) s -> p c s', c=2))

        # ---- o_proj: two passes over hd-chunk halves, accumulate in SBUF ----
        obuf = p_kv.tile([128, 8, 1024], BF16, tag='obuf', name='obuf')
        for bi in range(8):
            for nt in range(2):
                psO = ps_m.tile([128, 512], F32, tag='ps', name='psO')
                for c in range(8):
                    nc.tensor.matmul(psO[:], ctf0[c // 2][:, c % 2, bass.ts(bi, 128)],
                                     wo_s[c // 4][:, c % 4, bass.ts(nt, 512)],
                                     start=(c == 0), stop=(c == 7))
                nc.vector.tensor_scalar_add(obuf[:, bi, bass.ts(nt, 512)], psO[:], 0.0)
        for bi in range(8):
            for nt in range(2):
                psO = ps_m.tile([128, 512], F32, tag='ps', name='psO2')
                for c in range(8):
                    nc.tensor.matmul(psO[:], ctf1[c // 2][:, c % 2, bass.ts(bi, 128)],
                                     wo_s[2 + c // 4][:, c % 4, bass.ts(nt, 512)],
                                     start=(c == 0), stop=(c == 7))
                ob = p_ob.tile([128, 512], F32, tag='ob', name='ob')
                nc.vector.tensor_add(ob[:], psO[:], obuf[:, bi, bass.ts(nt, 512)])
                nc.sync.dma_start(out_e[bass.ts(bi, 128), bass.ts(nt, 512)], ob[:])

    split_multi_waits(nc)
    return nc


def mul_b(eng, out, a, b):
    """tensor_tensor multiply with free-dim broadcast of b over dim 1."""
    a2, b2 = bass.broadcast_tensor_aps(a, b)
    eng.tensor_mul(out, a2, b2)


# ---------------------------------------------------------------------------
_NC_CACHE = None
_LAST_IN_MAPS = None


def _get_nc():
    global _NC_CACHE
    if _NC_CACHE is None:
        _NC_CACHE = build_kernel()
    return _NC_CACHE


def kernel(hidden_states, cos, sin, q_w, k_w, v_w, o_w, q_norm_w, k_norm_w):
    from concourse.bass_utils import run_bass_kernel_spmd

    hidden_states = np.asarray(hidden_states, np.float32)
    cos = np.asarray(cos, np.float32)
    sin = np.asarray(sin, np.float32)
    q_w = np.asarray(q_w, np.float32)
    k_w = np.asarray(k_w, np.float32)
    v_w = np.asarray(v_w, np.float32)
    o_w = np.asarray(o_w, np.float32)
    q_norm_w = np.asarray(q_norm_w, np.float32)
    k_norm_w = np.asarray(k_norm_w, np.float32)

    tri_np = np.triu(np.ones((128, 128), np.float32))  # [sj,si]: valid sj<=si
    iden_np = np.eye(128, dtype=np.float32)
    operm = np.concatenate([np.arange(h * 128, (h + 1) * 128) for h in OHEAD_ORDER])

    def rope_tabs(c, s_, w):
        # tables [rows, 4, 64]: A=c_lo*w_lo, B=s_lo*w_hi, C=c_lo*w_hi, D=s_lo*w_lo
        cl, sl = c[:, 0:64], s_[:, 0:64]
        wl, wh = w[0:64], w[64:128]
        return np.stack([cl * wl, sl * wh, cl * wh, sl * wl], axis=1).astype(np.float32)

    in_maps = []
    for c in range(8):
        b, sh, hh = c >> 2, (c >> 1) & 1, c & 1
        blks = MYBLKS[sh]
        rows = np.concatenate([np.arange(g * 128, (g + 1) * 128) for g in blks])
        hT = np.ascontiguousarray(hidden_states[b][rows].T)
        qwT = np.ascontiguousarray(q_w[hh * 1024:(hh + 1) * 1024].T)
        kwT = np.ascontiguousarray(k_w[hh * 512:(hh + 1) * 512].T)
        vwT = np.ascontiguousarray(v_w[hh * 512:(hh + 1) * 512].T)
        owT = np.ascontiguousarray(o_w[hh * 1024:(hh + 1) * 1024].T[operm])
        qtab = rope_tabs(cos[b][rows], sin[b][rows], q_norm_w)
        ktab = rope_tabs(cos[b][rows], sin[b][rows], k_norm_w)
        # diagonal masks dm[bi, t]: t=0 -> sj block BOUNDS[bi]-1, t=1 -> BOUNDS[bi]
        dm = np.zeros((8, 2, 128, 128), np.float32)
        for bi in range(8):
            g, gb = blks[bi], BOUNDS[bi]
            for t, j in enumerate((gb - 1, gb)):
                if j < g:
                    dm[bi, t] = 1.0
                elif j == g:
                    dm[bi, t] = tri_np
                # j > g: stays 0 (block fully masked)
        in_maps.append(dict(
            hT=hT, qwT=qwT, kwT=kwT, vwT=vwT, owT=owT,
            qtab=qtab, ktab=ktab, iden=iden_np, dm=dm))

    global _LAST_IN_MAPS
    _LAST_IN_MAPS = in_maps
    nc = _get_nc()
    res = run_bass_kernel_spmd(nc, in_maps, core_ids=list(range(8)))

    out = np.zeros((B, S, HID), np.float32)
    for c in range(8):
        b, sh, hh = c >> 2, (c >> 1) & 1, c & 1
        o = res.results[c]['out']  # [1024, 1024]
        for bi, g in enumerate(MYBLKS[sh]):
            out[b, g * 128:(g + 1) * 128, hh * 1024:(hh + 1) * 1024] = \
                o[bi * 128:(bi + 1) * 128]
    return out


if __name__ == '__main__':
    sys.path.insert(0, '/root/problem')
    import reference
    inputs = {k: np.asarray(v) for k, v in reference.setup_inputs().items()}
    exp = np.asarray(reference.reference(**inputs))
    act = kernel(**inputs)
    err = np.abs(act - exp)
    rel = np.linalg.norm(act - exp) / np.linalg.norm(exp)
    print('Relative error:', rel, 'max abs err:', err.max())
